# revision 15
# baseline (speedup 1.0000x reference)
"""Trainium2 Bass kernel for nn_Attention_3032246911698 (sparse_attention).

Computes, per batch row b:
    score_dec = v[0] @ W_v.T + attn_b                      # [B, H]
    score_enc = einsum('ble,he->blh', encoder_out, W_e)    # [B, L, H]
    en        = tanh(score_dec[:,None,:] + score_enc)      # [B, L, H]
    att       = einsum('blh,h->bl', en, v_w[0])            # [B, L]
    att       = where(mask == 0, -1e10, att)
    out       = softmax(att, axis=1)                       # [B, L]

Sharding: data-parallel over batch B=16 across 8 NeuronCores (2 rows each).
Weights are replicated.  No cross-core communication is needed.

Device dataflow per core (Bc=2, L=2048, H=1024, E=2H=2048):
  - host pre-transposes/casts the small replicated weights:
      attn_wT  [3072, 1024] bf16  (rows 0:1024 = W_v.T, rows 1024:3072 = W_e.T)
      decT     [1024, Bc]   bf16, attn_b [1024,1] f32, v_w [1024,1] bf16
  - score_dec computed on-device with 64 tiny matmuls.
  - main loop: stream encoder_out in [128 tok, 2048 e] f32 tiles, cast to
    bf16 (ACT), transpose 128x128 blocks on the TensorEngine into PSUM,
    copy to SBUF (DVE) forming encT tiles [e=128, t=512]; then 16
    accumulating bf16 matmuls per h-chunk produce score[h=128, t=512] in
    PSUM; tanh+bias (ACT, bias = score_dec column) writes en bf16; 8 more
    matmuls against v_w reduce over h into att[1, t=512]; mask+softmax on
    DVE/ACT; store [Bc, 2048] f32.
"""

import os
import sys

import numpy as np

for _p in ("/opt/trn_rl_repo", "/root/.axon_site/_ro/trn_rl_repo"):
    if os.path.isdir(_p) and _p not in sys.path:
        sys.path.append(_p)

import concourse.bass as bass
import concourse.mybir as mybir
import concourse.tile as tile
from concourse import bacc
from concourse.bass_utils import run_bass_kernel_spmd
from concourse.masks import make_identity

try:
    import ml_dtypes

    BF16 = ml_dtypes.bfloat16
except ImportError:  # jax always ships ml_dtypes, but be safe
    import jax.numpy as jnp

    BF16 = jnp.bfloat16

F32 = mybir.dt.float32
BF = mybir.dt.bfloat16

N_CORES = 8
B, L, H = 16, 2048, 1024
E = 2 * H
BC = B // N_CORES          # 2 batch rows per core
TCH = 512                  # tokens per t-chunk
NSUB = TCH // 128          # 128-token subtiles per chunk
NCHUNK = L // TCH          # t-chunks per batch row
NEG_INF = -1.0e10


def build_nc():
    # Bacc (not raw Bass): its compile pipeline legalizes multi-wait sync via
    # event semaphores — walrus only accepts one sync-wait per instruction.
    nc = bacc.Bacc()

    enc = nc.declare_dram_parameter("encoder_out", [BC, L, E], F32, isOutput=False)
    # (mask-1)*1e10 precast to bf16: 0 where kept, ~-1e10 where masked; added
    # into the attention PSUM via a K=1 matmul so no tensor-tensor op needed.
    maskadd = nc.declare_dram_parameter("maskadd", [BC, L], BF, isOutput=False)
    wT = nc.declare_dram_parameter("attn_wT", [3 * H, H], BF, isOutput=False)
    decT = nc.declare_dram_parameter("decT", [H, BC], BF, isOutput=False)
    bcol = nc.declare_dram_parameter("attn_bT", [H, 1], F32, isOutput=False)
    vwcol = nc.declare_dram_parameter("v_wT", [H, 1], BF, isOutput=False)
    out = nc.declare_dram_parameter("out", [BC, L], F32, isOutput=True)

    KC = H // 128            # 8 h/hi chunks
    EC = E // 128            # 16 e chunks

    with tile.TileContext(nc) as tc:
        with (
            tc.tile_pool(name="consts", bufs=1) as consts,
            tc.tile_pool(name="nat", bufs=4) as nat_pool,
            tc.tile_pool(name="natb", bufs=3) as natb_pool,
            tc.tile_pool(name="encT", bufs=2) as encT_pool,
            tc.tile_pool(name="en", bufs=2) as en_pool,
            tc.tile_pool(name="rowbig", bufs=2) as rowbig_pool,
            tc.tile_pool(name="rowsmall", bufs=1) as rowsmall_pool,
            tc.tile_pool(name="psum_tp", bufs=3, space="PSUM") as tp_psum,
            tc.tile_pool(name="psum_score", bufs=2, space="PSUM") as score_psum,
            tc.tile_pool(name="psum_att", bufs=2, space="PSUM") as att_psum,
        ):
            # ---- constants / weights ---------------------------------------
            ident = consts.tile([128, 128], BF)
            make_identity(nc, ident)

            ones1 = consts.tile([1, 1], BF)
            nc.gpsimd.memset(ones1, 1.0)

            w_tile = consts.tile([128, 3 * KC, H], BF)      # [p, chunk, h_out]
            nc.sync.dma_start(w_tile, wT.rearrange("(c p) h -> p c h", p=128))

            dec_tile = consts.tile([128, KC, BC], BF)
            nc.sync.dma_start(dec_tile, decT.rearrange("(c p) b -> p c b", p=128))

            b_tile = consts.tile([128, KC, 1], F32)
            nc.sync.dma_start(b_tile, bcol.rearrange("(c p) o -> p c o", p=128))

            vw_tile = consts.tile([128, KC, 1], BF)
            nc.sync.dma_start(vw_tile, vwcol.rearrange("(c p) o -> p c o", p=128))

            # ---- score_dec = dec @ W_v.T + attn_b, stored transposed -------
            # sd_tile[:, hoc, b] = sum_hi W_vT[hi, ho] * dec[hi, b] + attn_b[ho]
            sd_tile = consts.tile([128, KC, BC], F32)
            for hoc in range(KC):
                ps_sd = att_psum.tile([128, BC], F32, tag="attps")
                for hic in range(KC):
                    nc.tensor.matmul(
                        ps_sd,
                        lhsT=w_tile[:, hic, hoc * 128:(hoc + 1) * 128],
                        rhs=dec_tile[:, hic, :],
                        start=(hic == 0),
                        stop=(hic == KC - 1),
                    )
                # ACT (not DVE tensor_scalar): the TensorScalarPtr ISA struct
                # only carries one sync-wait slot and this op needs two.
                nc.scalar.activation(
                    sd_tile[:, hoc, :],
                    ps_sd,
                    mybir.ActivationFunctionType.Identity,
                    bias=b_tile[:, hoc, :],
                )

            # ---- main loop --------------------------------------------------
            for b in range(BC):
                logits = rowbig_pool.tile([1, L], F32, tag="logits")
                maskb = rowsmall_pool.tile([1, L], BF, tag="maskb")
                nc.sync.dma_start(maskb, maskadd[b:b + 1, :])
                for tch in range(NCHUNK):
                    encT = encT_pool.tile([128, EC, TCH], BF)
                    for ts in range(NSUB):
                        t0 = tch * TCH + ts * 128
                        natf = nat_pool.tile([128, E], F32)
                        nc.sync.dma_start(natf, enc[b, t0:t0 + 128, :])
                        natb = natb_pool.tile([128, E], BF)
                        nc.scalar.copy(natb, natf)
                        for ec in range(EC):
                            ps_t = tp_psum.tile([128, 128], BF)
                            nc.tensor.transpose(
                                ps_t, natb[:, ec * 128:(ec + 1) * 128], ident
                            )
                            nc.vector.tensor_copy(
                                encT[:, ec, ts * 128:(ts + 1) * 128], ps_t
                            )

                    en_big = en_pool.tile([128, KC, TCH], BF)
                    for hc in range(KC):
                        ps_score = score_psum.tile([128, TCH], F32)
                        for ec in range(EC):
                            nc.tensor.matmul(
                                ps_score,
                                lhsT=w_tile[:, KC + ec, hc * 128:(hc + 1) * 128],
                                rhs=encT[:, ec, :],
                                start=(ec == 0),
                                stop=(ec == EC - 1),
                            )
                        nc.scalar.activation(
                            en_big[:, hc, :],
                            ps_score,
                            mybir.ActivationFunctionType.Tanh,
                            bias=sd_tile[:, hc, b:b + 1],
                        )

                    ps_att = att_psum.tile([1, TCH], F32, tag="attps")
                    for hc in range(KC):
                        nc.tensor.matmul(
                            ps_att,
                            lhsT=vw_tile[:, hc, :],
                            rhs=en_big[:, hc, :],
                            start=(hc == 0),
                            stop=False,
                        )
                    # += (mask-1)*1e10 as a K=1 rank-1 update: masked tokens
                    # drop to ~-1e10 with no elementwise mask op anywhere.
                    nc.tensor.matmul(
                        ps_att,
                        lhsT=ones1,
                        rhs=maskb[:, tch * TCH:(tch + 1) * TCH],
                        start=False,
                        stop=True,
                    )
                    nc.vector.tensor_copy(logits[:, tch * TCH:(tch + 1) * TCH], ps_att)

                # ---- softmax over L on a single partition row --------------
                mx = rowsmall_pool.tile([1, 1], F32, tag="mx")
                nc.vector.reduce_max(mx, logits, axis=mybir.AxisListType.X)
                negmx = rowsmall_pool.tile([1, 1], F32, tag="negmx")
                nc.scalar.mul(negmx, mx, -1.0)
                exps = rowsmall_pool.tile([1, L], F32, tag="exps")
                sumx = rowsmall_pool.tile([1, 1], F32, tag="sumx")
                nc.scalar.activation(
                    exps,
                    logits,
                    mybir.ActivationFunctionType.Exp,
                    bias=negmx[:, :],
                    accum_out=sumx,
                )
                rcp = rowsmall_pool.tile([1, 1], F32, tag="rcp")
                nc.vector.reciprocal(rcp, sumx)
                orow = rowbig_pool.tile([1, L], F32, tag="orow")
                nc.vector.tensor_scalar_mul(orow, exps, rcp[:, :])
                nc.sync.dma_start(out[b:b + 1, :], orow)

    nc.finalize()
    return nc


_NC_CACHE = None


def _get_nc():
    global _NC_CACHE
    if _NC_CACHE is None:
        _NC_CACHE = build_nc()
    return _NC_CACHE


def prepare_in_maps(encoder_out, mask, v, attn_w, attn_b, v_w):
    encoder_out = np.ascontiguousarray(np.asarray(encoder_out, dtype=np.float32))
    maskadd = ((np.asarray(mask, dtype=np.float32) - 1.0) * 1.0e10).astype(BF16)
    wTb = np.ascontiguousarray(np.asarray(attn_w, dtype=np.float32).T).astype(BF16)
    decTb = np.ascontiguousarray(np.asarray(v[0], dtype=np.float32).T).astype(BF16)
    bcol = np.ascontiguousarray(np.asarray(attn_b, dtype=np.float32).reshape(H, 1))
    vwcol = np.ascontiguousarray(
        np.asarray(v_w, dtype=np.float32).reshape(H, 1)
    ).astype(BF16)

    in_maps = []
    for c in range(N_CORES):
        s = slice(c * BC, (c + 1) * BC)
        in_maps.append(
            {
                "encoder_out": encoder_out[s],
                "maskadd": maskadd[s],
                "attn_wT": wTb,
                "decT": np.ascontiguousarray(decTb[:, s]),
                "attn_bT": bcol,
                "v_wT": vwcol,
            }
        )
    return in_maps


def run(inputs, trace=False):
    nc = _get_nc()
    in_maps = prepare_in_maps(**inputs)
    res = run_bass_kernel_spmd(nc, in_maps, core_ids=list(range(N_CORES)), trace=trace)
    out = np.concatenate([res.results[c]["out"] for c in range(N_CORES)], axis=0)
    return out.astype(np.float32), res


def kernel(**inputs):
    out, _ = run(inputs, trace=False)
    return out


# revision 18
# speedup vs baseline: 1.0218x; 1.0218x over previous
"""Trainium2 Bass kernel for nn_Attention_3032246911698 (sparse_attention).

Computes, per batch row b:
    score_dec = v[0] @ W_v.T + attn_b                      # [B, H]
    score_enc = einsum('ble,he->blh', encoder_out, W_e)    # [B, L, H]
    en        = tanh(score_dec[:,None,:] + score_enc)      # [B, L, H]
    att       = einsum('blh,h->bl', en, v_w[0])            # [B, L]
    att       = where(mask == 0, -1e10, att)
    out       = softmax(att, axis=1)                       # [B, L]

Sharding: data-parallel over batch B=16 across 8 NeuronCores (2 rows each).
Weights are replicated.  No cross-core communication is needed.

Device dataflow per core (Bc=2, L=2048, H=1024, E=2H=2048):
  - host pre-transposes/casts the small replicated weights:
      attn_wT  [3072, 1024] bf16  (rows 0:1024 = W_v.T, rows 1024:3072 = W_e.T)
      decT     [1024, Bc]   bf16, attn_b [1024,1] f32, v_w [1024,1] bf16
  - score_dec computed on-device with 64 tiny matmuls.
  - main loop: stream encoder_out in [128 tok, 2048 e] f32 tiles, cast to
    bf16 (ACT), transpose 128x128 blocks on the TensorEngine into PSUM,
    copy to SBUF (DVE) forming encT tiles [e=128, t=512]; then 16
    accumulating bf16 matmuls per h-chunk produce score[h=128, t=512] in
    PSUM; tanh+bias (ACT, bias = score_dec column) writes en bf16; 8 more
    matmuls against v_w reduce over h into att[1, t=512]; mask+softmax on
    DVE/ACT; store [Bc, 2048] f32.
"""

import os
import sys

import numpy as np

for _p in ("/opt/trn_rl_repo", "/root/.axon_site/_ro/trn_rl_repo"):
    if os.path.isdir(_p) and _p not in sys.path:
        sys.path.append(_p)

import concourse.bass as bass
import concourse.mybir as mybir
import concourse.tile as tile
from concourse import bacc
from concourse.bass_utils import run_bass_kernel_spmd

try:
    import ml_dtypes

    BF16 = ml_dtypes.bfloat16
except ImportError:  # jax always ships ml_dtypes, but be safe
    import jax.numpy as jnp

    BF16 = jnp.bfloat16

F32 = mybir.dt.float32
BF = mybir.dt.bfloat16

N_CORES = 8
B, L, H = 16, 2048, 1024
E = 2 * H
BC = B // N_CORES          # 2 batch rows per core
TCH = 512                  # tokens per t-chunk
NSUB = TCH // 128          # 128-token subtiles per chunk
NCHUNK = L // TCH          # t-chunks per batch row
NEG_INF = -1.0e10


def build_nc():
    # Bacc (not raw Bass): its compile pipeline legalizes multi-wait sync via
    # event semaphores — walrus only accepts one sync-wait per instruction.
    nc = bacc.Bacc()

    enc = nc.declare_dram_parameter("encoder_out", [BC, L, E], F32, isOutput=False)
    # (mask-1)*1e10 precast to bf16: 0 where kept, ~-1e10 where masked; added
    # into the attention PSUM via a K=1 matmul so no tensor-tensor op needed.
    maskadd = nc.declare_dram_parameter("maskadd", [BC, L], BF, isOutput=False)
    wT = nc.declare_dram_parameter("attn_wT", [3 * H, H], BF, isOutput=False)
    decT = nc.declare_dram_parameter("decT", [H, BC], BF, isOutput=False)
    bcol = nc.declare_dram_parameter("attn_bT", [H, 1], F32, isOutput=False)
    vwcol = nc.declare_dram_parameter("v_wT", [H, 1], BF, isOutput=False)
    out = nc.declare_dram_parameter("out", [BC, L], F32, isOutput=True)

    KC = H // 128            # 8 h/hi chunks
    EC = E // 128            # 16 e chunks

    with tile.TileContext(nc) as tc:
        with (
            tc.tile_pool(name="consts", bufs=1) as consts,
            tc.tile_pool(name="natb", bufs=6) as natb_pool,
            tc.tile_pool(name="encT", bufs=2) as encT_pool,
            tc.tile_pool(name="en", bufs=2) as en_pool,
            tc.tile_pool(name="rowbig", bufs=2) as rowbig_pool,
            tc.tile_pool(name="rowsmall", bufs=1) as rowsmall_pool,
            tc.tile_pool(name="psum_score", bufs=4, space="PSUM") as score_psum,
            tc.tile_pool(name="psum_att", bufs=2, space="PSUM") as att_psum,
        ):
            # ---- constants / weights ---------------------------------------
            ones1 = consts.tile([1, 1], BF)
            nc.gpsimd.memset(ones1, 1.0)

            w_tile = consts.tile([128, 3 * KC, H], BF)      # [p, chunk, h_out]
            nc.sync.dma_start(w_tile, wT.rearrange("(c p) h -> p c h", p=128))

            dec_tile = consts.tile([128, KC, BC], BF)
            nc.sync.dma_start(dec_tile, decT.rearrange("(c p) b -> p c b", p=128))

            b_tile = consts.tile([128, KC, 1], F32)
            nc.sync.dma_start(b_tile, bcol.rearrange("(c p) o -> p c o", p=128))

            vw_tile = consts.tile([128, KC, 1], BF)
            nc.sync.dma_start(vw_tile, vwcol.rearrange("(c p) o -> p c o", p=128))

            # ---- score_dec = dec @ W_v.T + attn_b, stored transposed -------
            # sd_tile[:, hoc, b] = sum_hi W_vT[hi, ho] * dec[hi, b] + attn_b[ho]
            sd_tile = consts.tile([128, KC, BC], F32)
            for hoc in range(KC):
                ps_sd = att_psum.tile([128, BC], F32, tag="attps")
                for hic in range(KC):
                    nc.tensor.matmul(
                        ps_sd,
                        lhsT=w_tile[:, hic, hoc * 128:(hoc + 1) * 128],
                        rhs=dec_tile[:, hic, :],
                        start=(hic == 0),
                        stop=(hic == KC - 1),
                    )
                # ACT (not DVE tensor_scalar): the TensorScalarPtr ISA struct
                # only carries one sync-wait slot and this op needs two.
                nc.scalar.activation(
                    sd_tile[:, hoc, :],
                    ps_sd,
                    mybir.ActivationFunctionType.Identity,
                    bias=b_tile[:, hoc, :],
                )

            # ---- main loop --------------------------------------------------
            for b in range(BC):
                logits = rowbig_pool.tile([1, L], F32, tag="logits")
                maskb = rowsmall_pool.tile([1, L], BF, tag="maskb")
                nc.sync.dma_start(maskb, maskadd[b:b + 1, :])
                for tch in range(NCHUNK):
                    encT = encT_pool.tile([128, EC, TCH], BF)
                    for ts in range(NSUB):
                        t0 = tch * TCH + ts * 128
                        # SWDGE cast-DMA: f32 DRAM -> bf16 SBUF in flight
                        natb = natb_pool.tile([128, E], BF)
                        nc.gpsimd.dma_start(natb, enc[b, t0:t0 + 128, :])
                        # xbar transpose (SBUF->SBUF): one DMA turns the whole
                        # [128 tok, 2048 e] tile into 16 [e=128, t=128] blocks
                        # laid down inside encT's [128, EC, TCH] layout.
                        nc.sync.dma_start(
                            encT[:, :, ts * 128:(ts + 1) * 128],
                            natb[:, :],
                            transpose=True,
                        )

                    en_big = en_pool.tile([128, KC, TCH], BF)
                    for hc in range(KC):
                        ps_score = score_psum.tile([128, TCH], F32)
                        for ec in range(EC):
                            nc.tensor.matmul(
                                ps_score,
                                lhsT=w_tile[:, KC + ec, hc * 128:(hc + 1) * 128],
                                rhs=encT[:, ec, :],
                                start=(ec == 0),
                                stop=(ec == EC - 1),
                            )
                        nc.scalar.activation(
                            en_big[:, hc, :],
                            ps_score,
                            mybir.ActivationFunctionType.Tanh,
                            bias=sd_tile[:, hc, b:b + 1],
                        )

                    ps_att = att_psum.tile([1, TCH], F32, tag="attps")
                    for hc in range(KC):
                        nc.tensor.matmul(
                            ps_att,
                            lhsT=vw_tile[:, hc, :],
                            rhs=en_big[:, hc, :],
                            start=(hc == 0),
                            stop=False,
                        )
                    # += (mask-1)*1e10 as a K=1 rank-1 update: masked tokens
                    # drop to ~-1e10 with no elementwise mask op anywhere.
                    nc.tensor.matmul(
                        ps_att,
                        lhsT=ones1,
                        rhs=maskb[:, tch * TCH:(tch + 1) * TCH],
                        start=False,
                        stop=True,
                    )
                    nc.vector.tensor_copy(logits[:, tch * TCH:(tch + 1) * TCH], ps_att)

                # ---- softmax over L on a single partition row --------------
                mx = rowsmall_pool.tile([1, 1], F32, tag="mx")
                nc.vector.reduce_max(mx, logits, axis=mybir.AxisListType.X)
                negmx = rowsmall_pool.tile([1, 1], F32, tag="negmx")
                nc.scalar.mul(negmx, mx, -1.0)
                exps = rowsmall_pool.tile([1, L], F32, tag="exps")
                sumx = rowsmall_pool.tile([1, 1], F32, tag="sumx")
                nc.scalar.activation(
                    exps,
                    logits,
                    mybir.ActivationFunctionType.Exp,
                    bias=negmx[:, :],
                    accum_out=sumx,
                )
                rcp = rowsmall_pool.tile([1, 1], F32, tag="rcp")
                nc.vector.reciprocal(rcp, sumx)
                orow = rowbig_pool.tile([1, L], F32, tag="orow")
                nc.vector.tensor_scalar_mul(orow, exps, rcp[:, :])
                nc.sync.dma_start(out[b:b + 1, :], orow)

    nc.finalize()
    return nc


_NC_CACHE = None


def _get_nc():
    global _NC_CACHE
    if _NC_CACHE is None:
        _NC_CACHE = build_nc()
    return _NC_CACHE


def prepare_in_maps(encoder_out, mask, v, attn_w, attn_b, v_w):
    encoder_out = np.ascontiguousarray(np.asarray(encoder_out, dtype=np.float32))
    maskadd = ((np.asarray(mask, dtype=np.float32) - 1.0) * 1.0e10).astype(BF16)
    wTb = np.ascontiguousarray(np.asarray(attn_w, dtype=np.float32).T).astype(BF16)
    decTb = np.ascontiguousarray(np.asarray(v[0], dtype=np.float32).T).astype(BF16)
    bcol = np.ascontiguousarray(np.asarray(attn_b, dtype=np.float32).reshape(H, 1))
    vwcol = np.ascontiguousarray(
        np.asarray(v_w, dtype=np.float32).reshape(H, 1)
    ).astype(BF16)

    in_maps = []
    for c in range(N_CORES):
        s = slice(c * BC, (c + 1) * BC)
        in_maps.append(
            {
                "encoder_out": encoder_out[s],
                "maskadd": maskadd[s],
                "attn_wT": wTb,
                "decT": np.ascontiguousarray(decTb[:, s]),
                "attn_bT": bcol,
                "v_wT": vwcol,
            }
        )
    return in_maps


def run(inputs, trace=False):
    nc = _get_nc()
    in_maps = prepare_in_maps(**inputs)
    res = run_bass_kernel_spmd(nc, in_maps, core_ids=list(range(N_CORES)), trace=trace)
    out = np.concatenate([res.results[c]["out"] for c in range(N_CORES)], axis=0)
    return out.astype(np.float32), res


def kernel(**inputs):
    out, _ = run(inputs, trace=False)
    return out
